# revision 48
# baseline (speedup 1.0000x reference)
"""Multi-head causal self-attention (B=4, T=2048, C=1024, H=16) on 8 TRN2 cores.

Sharding: core pair {2b, 2b+1} owns batch b; even core computes heads 0-7,
odd core heads 8-15 (tensor parallel over heads). Each core:
  1. qkvT projection from host-prepacked xT (bf16 matmuls, fp32 PSUM),
     emitted incrementally per 512-wide T-chunk; inputs are repacked on
     the host into per-partition-contiguous layouts, and the first DMA
     descriptors carry exactly the pieces the first q/k chains need (the
     DMA rings drain ~2 descriptors per ~4.5us round under 8-core HBM
     contention, so descriptor count on the critical path dominates)
  2. causal attention processed chunk 0 -> 3 in scoresT [Tk, Tq]
     orientation; per k-tile: two heads' score matmuls packed into disjoint
     PE row groups (concurrent via tile_position), one exp per pair on
     ScalarE (scale=1/8), AV^T matmuls with an appended ones column
     producing softmax denominators for free.  Diagonal k-tiles compute
     only the un-masked column range (partial-N); the shared [128,128]
     triangle mask runs on the otherwise-idle GpSimd engine.
     The m-loop is software-pipelined: scores+exp for m+1 are emitted
     ahead of m's fillers and AV matmuls, and each call's first
     scores+exp are emitted before the previous call's last AVs, so
     ScalarE (the global pacer at ~1.1us per full-width exp) never waits
     behind filler matmuls or DVE evacuations in the in-order queues
  3. softmax: denominator rows staged through [1,512] SBUF tiles to a
     head-major [128, c, 128] sums tile (pair j based at partition 32j
     for DVE alignment); per-pair [8,128] reciprocals and 4x N=128
     selector-matmul broadcasts + in-place multiply run as fillers right
     after each pair's attn call, leaving only pair 3's softmax in the
     tail (covered by pre-opened j2<3 c_proj partials)
  4. partial c_proj over local 512 channels; proj bias (b_proj/2 +
     bv@W_proj/2 folded into one row, broadcast once into a [128,C] SBUF
     tile via a single ones-matmul) added on the DVE during PSUM
     evacuation
  5. per-tile partial outputs DMA'd straight to DRAM as produced; the
     host sums each core pair's partials (no device collectives at all).
Every qkv/proj chain is injected as a filler into the exp-paced attention
m-loops at an explicit position chosen so each call's filler stream-time
matches its exp-paced hole, keeping the PE near its issue roofline
without re-throttling (HAM).
"""

import math
import numpy as np
import ml_dtypes
from contextlib import ExitStack

import concourse.bass as bass
import concourse.tile as tile
from concourse import bacc, mybir
from concourse.bass_utils import run_bass_kernel_spmd

bf16 = ml_dtypes.bfloat16
F32 = mybir.dt.float32
BF16 = mybir.dt.bfloat16
AF = mybir.ActivationFunctionType
ADD = mybir.AluOpType.add

B, T, C, H = 4, 2048, 1024, 16
D = C // H              # 64 head dim
NCORES = 8
HL = H // 2             # 8 heads per core
CL = HL * D             # 512 local channels

_CACHE = {}


def _build():
    nc = bacc.Bacc("TRN2", target_bir_lowering=False, debug=False,
                   num_devices=NCORES)

    xt_d = nc.dram_tensor("xt", [128, 4, 8, 512], BF16, kind="ExternalInput").ap()
    waq0_d = nc.dram_tensor("waq0", [128, 8, 128], BF16, kind="ExternalInput").ap()
    wak0_d = nc.dram_tensor("wak0", [128, 8, 128], BF16, kind="ExternalInput").ap()
    wav_d = nc.dram_tensor("wav", [128, 8, 512], BF16, kind="ExternalInput").ap()
    waq13_d = nc.dram_tensor("waq13", [128, 3, 8, 128], BF16, kind="ExternalInput").ap()
    wak13_d = nc.dram_tensor("wak13", [128, 3, 8, 128], BF16, kind="ExternalInput").ap()
    wp_d = nc.dram_tensor("wp", [128, 4, 1024], BF16, kind="ExternalInput").ap()
    bqk_d = nc.dram_tensor("bqk", [128, 8], F32, kind="ExternalInput").ap()
    bye_d = nc.dram_tensor("bye", [1, C], BF16, kind="ExternalInput").ap()
    tri_d = nc.dram_tensor("tri", [128, 1, 128], BF16, kind="ExternalInput").ap()
    sel_d = nc.dram_tensor("sel", [128, 16, 128], BF16, kind="ExternalInput").ap()
    y_d = nc.dram_tensor("y", [16, 128, C], BF16, kind="ExternalOutput").ap()

    with tile.TileContext(nc) as tc, ExitStack() as ctx:
        cst = ctx.enter_context(tc.tile_pool(name="cst", bufs=1))
        work = ctx.enter_context(tc.tile_pool(name="work", bufs=16))
        ysb_p = ctx.enter_context(tc.tile_pool(name="ysb", bufs=2))
        stg_p = ctx.enter_context(tc.tile_pool(name="stg", bufs=4))
        ps_mm = ctx.enter_context(tc.tile_pool(name="psmm", bufs=2, space="PSUM"))
        ps_s = ctx.enter_context(tc.tile_pool(name="pss", bufs=2, space="PSUM"))
        ps_av = ctx.enter_context(tc.tile_pool(name="psav", bufs=2, space="PSUM"))

        # ---- persistent SBUF tensors ----
        xT = cst.tile([128, 4, 8, 512], BF16)   # x^T  [p, ch, ko, t']
        waq0 = cst.tile([128, 8, 128], BF16)
        wak0 = cst.tile([128, 8, 128], BF16)
        wav = cst.tile([128, 8, 512], BF16)
        waq13 = cst.tile([128, 3, 8, 128], BF16)  # pair-major: [j-1, ko, :]
        wak13 = cst.tile([128, 3, 8, 128], BF16)
        wp = cst.tile([128, 4, C], BF16)        # W_proj local rows
        bqk = cst.tile([128, 8], F32)           # q/k biases per qT/kT tile
        bye = cst.tile([1, C], BF16)            # (b_proj + bv@W_proj)/2
        byeb = cst.tile([128, C], BF16)         # ... broadcast to all parts
        tri = cst.tile([128, 1, 128], BF16)     # shared diagonal triangle mask
        ones = cst.tile([1, 128], BF16)
        sel = cst.tile([128, 16, 128], BF16)    # per-(j,i) selectors
        qT = cst.tile([128, 4, T], BF16)
        kT = cst.tile([128, 4, T], BF16)
        vaug = cst.tile([128, 16, HL, D + 1], BF16)  # v rows + ones col
        avT = cst.tile([128, 4, T], BF16)       # attn-out^T (raw, then normed)
        sums = cst.tile([128, 4, 128], F32)     # denoms [32j+4hh+i, c, tq/4]
        rsum = cst.tile([128, 4, 128], BF16)    # their reciprocals

        # ---- input DMAs.  dma_start blocks the issuing engine's queue for
        # roughly the transfer time, so bulk loads go on sync + gpsimd
        # (idle until mid-kernel); scalar's ring starts earliest and takes
        # the small first-needed pieces, then stays free for exp.
        # all six descriptors the first scores depend on spread across the
        # THREE DMA rings' first rounds (each ring drains ~2 descriptors
        # per ~4.5us round under 8-core HBM contention)
        nc.sync.dma_start(waq0[:], waq0_d)
        nc.sync.dma_start(xT[:, 0, 0:2], xt_d[:, 0, 0:2])
        nc.sync.dma_start(wav[:, 0:4], wav_d[:, 0:4])
        nc.sync.dma_start(wav[:, 4:8], wav_d[:, 4:8])
        nc.sync.dma_start(xT[:, 1, 0:4], xt_d[:, 1, 0:4])
        nc.sync.dma_start(xT[:, 1, 4:8], xt_d[:, 1, 4:8])
        nc.scalar.dma_start(bqk[:], bqk_d)
        nc.scalar.dma_start(wak0[:], wak0_d)
        nc.scalar.dma_start(xT[:, 0, 2:4], xt_d[:, 0, 2:4])
        nc.scalar.dma_start(tri[:], tri_d)
        nc.gpsimd.dma_start(xT[:, 0, 4:6], xt_d[:, 0, 4:6])
        nc.gpsimd.dma_start(xT[:, 0, 6:8], xt_d[:, 0, 6:8])
        nc.gpsimd.dma_start(waq13[:, 0], waq13_d[:, 0])
        nc.gpsimd.dma_start(wak13[:, 0], wak13_d[:, 0])
        nc.gpsimd.dma_start(waq13[:, 1:3], waq13_d[:, 1:3])
        nc.gpsimd.dma_start(wak13[:, 1:3], wak13_d[:, 1:3])
        nc.gpsimd.dma_start(xT[:, 2], xt_d[:, 2])
        nc.gpsimd.dma_start(wp[:], wp_d)
        nc.gpsimd.dma_start(xT[:, 3], xt_d[:, 3])
        nc.vector.memset(ones[:], 1.0)
        nc.vector.memset(vaug[:, :, :, D], 1.0)
        nc.vector.memset(rsum[:], 0.0)
        nc.sync.dma_start(sel[:], sel_d)
        nc.sync.dma_start(bye[:], bye_d)

        def wa_q(ko, j):
            return waq0[:, ko, :] if j == 0 else waq13[:, j - 1, ko, :]

        def wa_k(ko, j):
            return wak0[:, ko, :] if j == 0 else wak13[:, j - 1, ko, :]

        # ---- QKV projection chains ----
        # qT/kT tile j holds heads {2j, 2j+1}.  qT = W_q^T @ x^T.
        def qk_chain(qk, j, ch):
            dst = qT if qk == 0 else kT
            ps = ps_mm.tile([128, 512], F32, tag="mm")
            for ko in range(8):
                nc.tensor.matmul(
                    ps[:],
                    lhsT=wa_q(ko, j) if qk == 0 else wa_k(ko, j),
                    rhs=xT[:, ch, ko, :],
                    start=(ko == 0), stop=(ko == 7))
            nc.vector.tensor_tensor(
                dst[:, j, 512 * ch:512 * (ch + 1)], ps[:],
                bqk[:, 4 * qk + j:4 * qk + j + 1].to_broadcast((128, 512)),
                ADD)

        # v in natural [T, c_local] layout, interleaved with ones columns
        def v_chain(m):
            ps = ps_mm.tile([128, 512], F32, tag="mm")
            for ko in range(8):
                nc.tensor.matmul(
                    ps[:],
                    lhsT=xT[:, m // 4, ko, 128 * (m % 4):128 * (m % 4) + 128],
                    rhs=wav[:, ko, :],
                    start=(ko == 0), stop=(ko == 7))
            nc.vector.tensor_copy(
                vaug[:, m, :, 0:D],
                ps[:].rearrange("p (h d) -> p h d", d=D))

        # one-time broadcast of the proj-bias row across all 128 partitions
        def bye_bcast(n):
            ps = ps_mm.tile([128, 512], F32, tag="mm")
            nc.tensor.matmul(ps[:], lhsT=ones[0:1, :],
                             rhs=bye[0:1, 512 * n:512 * (n + 1)],
                             start=True, stop=True)
            nc.vector.tensor_copy(byeb[:, 512 * n:512 * (n + 1)], ps[:])

        def recip_pair(c, j):
            """Head-pair reciprocal: partitions 32j..32j+8 hold pair j's
            denominator quarters, so each piece stays [8,128]-cheap AND
            is ready right after attn(c, j)."""
            with nc.allow_low_precision(reason="softmax reciprocal in bf16"):
                nc.vector.reciprocal(rsum[32 * j:32 * j + 8, c, :],
                                     sums[32 * j:32 * j + 8, c, :])

        def norm_mult(c, j, pool=None, use_pss=False):
            """Broadcast rsum rows {2j,2j+1} over 64 partitions each and
            normalize avT in place (deferred from attn(j, c))."""
            if use_pss:
                # tail-only: attention is done, so a score-PSUM buffer is
                # free and both ps_av banks can hold pre-opened proj chains
                pbt = ps_s.tile([128, 2, 512], F32, tag="s")
                pbc = pbt[:, 0, :]
            else:
                pbc = (pool or ps_mm).tile([128, 512], F32,
                                           tag="av" if pool else "mm")
            for i in range(4):
                nc.tensor.matmul(pbc[:, 128 * i:128 * (i + 1)],
                                 lhsT=sel[:, 4 * j + i, :], rhs=rsum[:, c, :],
                                 start=True, stop=True)
            nc.vector.tensor_mul(
                avT[:, j, 512 * c:512 * (c + 1)],
                avT[:, j, 512 * c:512 * (c + 1)], pbc[:])

        ysb_tiles = {}

        def proj_start(mt, n, pool=None, nj=4):
            """First nj of the 4 j-pair contractions of one 512-col n-chunk
            of c_proj for T-tile mt.  Splitting lets the j2<3 partials cover
            the tail's softmax latency (they don't need norm(c, 3))."""
            if n == 0:
                ysb_tiles[mt] = ysb_p.tile([128, C], BF16, tag="y",
                                           name=f"ysb{mt}")
            ps = (pool or ps_mm).tile([128, 512], F32,
                                      tag="av" if pool else "mm")
            for j2 in range(nj):
                nc.tensor.matmul(
                    ps[:],
                    lhsT=avT[:, j2, 128 * mt:128 * (mt + 1)],
                    rhs=wp[:, j2, 512 * n:512 * (n + 1)],
                    start=(j2 == 0), stop=(j2 == 3))
            return ps

        def proj_finish(mt, n, ps, nj0=4, split_dma=False):
            ysb = ysb_tiles[mt]
            for j2 in range(nj0, 4):
                nc.tensor.matmul(
                    ps[:],
                    lhsT=avT[:, j2, 128 * mt:128 * (mt + 1)],
                    rhs=wp[:, j2, 512 * n:512 * (n + 1)],
                    start=False, stop=(j2 == 3))
            nc.vector.tensor_tensor(
                ysb[:, 512 * n:512 * (n + 1)], ps[:],
                byeb[:, 512 * n:512 * (n + 1)], ADD)
            if split_dma:
                nc.sync.dma_start(y_d[mt, :, 512 * n:512 * (n + 1)],
                                  ysb[:, 512 * n:512 * (n + 1)])
                if n == 1:
                    del ysb_tiles[mt]
            elif n == 1:
                del ysb_tiles[mt]
                nc.sync.dma_start(y_d[mt], ysb[:])

        def proj_chain2(mt, n):
            proj_finish(mt, n, proj_start(mt, n), split_dma=True)

        def proj_chain(mt, n):
            proj_finish(mt, n, proj_start(mt, n))

        def attn_scores(j, c, m):
            """Scores + exp + diag-mask for one m-tile; returns (ex, o)."""
            s = m - 4 * c           # >=0: diagonal tile index
            o = 128 * s if s > 0 else 0
            pss = ps_s.tile([128, 2, 512], F32, tag="s")
            for hh in range(2):
                ro = hh * 64
                nc.tensor.matmul(
                    pss[:, hh, o:512],
                    lhsT=kT[ro:ro + 64, j, 128 * m:128 * (m + 1)],
                    rhs=qT[ro:ro + 64, j, 512 * c + o:512 * (c + 1)],
                    start=True, stop=True)
            ex = work.tile([128, 2, 512], BF16, tag="expT")
            nc.scalar.activation(ex[:, :, o:512], pss[:, :, o:512],
                                 AF.Exp, scale=1.0 / math.sqrt(D))
            if s >= 0:
                # triangle mask on the otherwise-idle GpSimd engine so the
                # in-order DVE queue never gates the AV matmuls
                nc.gpsimd.tensor_mul(
                    ex[:, :, o:o + 128], ex[:, :, o:o + 128],
                    tri[:].to_broadcast((128, 2, 128)))
            return ex, o

        def attn(j, c, fillers, head=None, nxt=None):
            """Head pair {2j, 2j+1}, Tq chunk c.  fillers: (pos, closure)
            pairs.  The m-loop is software-pipelined one step: scores+exp
            for m+1 are emitted ahead of m's fillers and AV matmuls, so
            ScalarE (the loop pacer) always sees its next exp immediately.
            `head` is this call's pre-emitted m=0 stage (from the previous
            call); the next call's m=0 stage is emitted before our last AV
            and returned, overlapping the inter-call pipeline bubble."""
            fillers = sorted(fillers, key=lambda pf: pf[0])
            nf = len(fillers)
            ntk = 4 * (c + 1)
            pos = [p for p, _ in fillers]
            fi = 0
            pavA = ps_av.tile([128, 512], F32, tag="av")
            pavB = ps_av.tile([128, 512], F32, tag="av")
            pend = []               # (m, ex, o) awaiting AV emission
            nh = None
            for m in range(ntk + 1):
                if m < ntk:
                    if m == 0 and head is not None:
                        pend.append((0,) + head)
                    else:
                        pend.append((m,) + attn_scores(j, c, m))
                while fi < nf and pos[fi] <= m - 1:
                    fillers[fi][1]()
                    fi += 1
                if m == ntk - 1 and nxt is not None:
                    # next call's first scores+exp one iteration early:
                    # they complete during this call's last exp, so the
                    # successor's exp fires back-to-back with ours (all its
                    # qT/kT producers are fillers at pos <= ntk-2, already
                    # flushed above, so the PE FIFO cannot deadlock)
                    nh = attn_scores(nxt[0], nxt[1], 0)
                if m >= 1:
                    m0, ex0, o0 = pend.pop(0)
                    for hh in range(2):
                        nc.tensor.matmul(
                            (pavA if hh == 0 else pavB)[0:D + 1, o0:512],
                            lhsT=vaug[:, m0, 2 * j + hh, :],
                            rhs=ex0[:, hh, o0:512],
                            start=(m0 == 0), stop=(m0 == ntk - 1))
            while fi < nf:
                fillers[fi][1]()
                fi += 1
            last = (c == 3 and j == 3)
            order = (0, 1) if last else (1, 0)
            for hh in range(2):
                pav = pavA if hh == 0 else pavB
                ro = hh * 64
                stg = stg_p.tile([1, 512], F32, tag="rs")
                for step in order:
                    if step == 0:
                        # denominator staging (feeds the deferred recip)
                        nc.vector.tensor_copy(stg[:], pav[D:D + 1, :])
                        h4 = 32 * j + 4 * hh
                        nc.sync.dma_start(sums[h4:h4 + 4, c, :], stg[:])
                    else:
                        # avT evacuation first mid-kernel: it frees the pav
                        # bank sooner, so the next call's first AV matmul
                        # (and with it ScalarE) restarts earlier.  The last
                        # call instead wants the sums DMA first (recip is
                        # its critical path).
                        nc.vector.tensor_copy(
                            avT[ro:ro + 64, j, 512 * c:512 * (c + 1)],
                            pav[0:D, :])
            return nh

        # ---- filler schedules: (pos, closure) per attn call ----
        def f_qk(qk, j, ch):
            return lambda: qk_chain(qk, j, ch)

        def f_v(m):
            return lambda: v_chain(m)

        def f_nm(c, j):
            return lambda: norm_mult(c, j)

        def f_rp(c, j):
            return lambda: recip_pair(c, j)

        def f_proj(mt, n):
            return lambda: proj_chain(mt, n)

        def f_bye(n):
            return lambda: bye_bcast(n)

        # Placement policy: every call's filler stream-time ~= its exp-paced
        # hole (ntk * ~0.57us).  q-chains land one call before their
        # consumer; k-chains and v-chains land INSIDE their first consumer
        # call (their k-tiles are only read from m-iter 4c on); proj spreads
        # late (it is the only work legal in the next chunk), leaving the
        # final chunk-3 calls as filled as the dependencies allow.
        fill = {(c, j): [] for c in range(4) for j in range(4)}
        fill[(0, 0)] += [(0, f_qk(0, 1, 0)), (0, f_v(0)), (0, f_qk(1, 1, 0)),
                         (1, f_v(1)), (1, f_v(2)), (2, f_v(3))]
        fill[(0, 1)] += [(0, f_qk(0, 2, 0)), (1, f_qk(1, 2, 0))]
        fill[(0, 2)] += [(0, f_qk(0, 3, 0)), (1, f_qk(1, 3, 0)),
                         (2, f_bye(0)), (2, f_bye(1))]
        fill[(0, 3)] += [(0, f_qk(0, 0, 1)), (1, f_v(6)), (2, f_v(7))]
        fill[(1, 0)] += [(0, f_qk(1, 0, 1)), (0, f_v(4)), (1, f_v(5)),
                         (3, f_qk(0, 1, 1))] + \
            [(4 + jj, f_rp(0, jj)) for jj in range(4)] + [(6, f_nm(0, 0))]
        fill[(1, 1)] += [(0, f_qk(1, 1, 1)), (2, f_qk(0, 2, 1)),
                         (3, f_nm(0, 1)), (4, f_nm(0, 2)), (6, f_nm(0, 3))]
        fill[(1, 2)] += [(0, f_qk(1, 2, 1)), (2, f_qk(0, 3, 1)),
                         (4, f_proj(0, 0)), (6, f_proj(0, 1))]
        fill[(1, 3)] += [(0, f_qk(1, 3, 1)), (2, f_qk(0, 0, 2)),
                         (4, f_v(11)), (6, f_proj(1, 0))]
        fill[(2, 0)] += [(0, f_qk(1, 0, 2)), (0, f_v(8)), (1, f_v(9)),
                         (2, f_qk(0, 1, 2)), (4, f_v(10))] + \
            [(4 + jj, f_rp(1, jj)) for jj in range(4)] + \
            [(6, f_nm(1, 0)), (8, f_nm(1, 1)), (10, f_proj(1, 1))]
        fill[(2, 1)] += [(0, f_qk(1, 1, 2)), (2, f_qk(0, 2, 2)),
                         (4, f_nm(1, 2)), (5, f_nm(1, 3)),
                         (6, f_proj(2, 0)), (8, f_proj(2, 1)),
                         (10, f_proj(4, 0))]
        fill[(2, 2)] += [(0, f_qk(1, 2, 2)), (2, f_qk(0, 3, 2)),
                         (4, f_proj(4, 1)), (5, f_proj(5, 0)),
                         (6, f_proj(5, 1)), (8, f_proj(3, 0)),
                         (10, f_proj(3, 1))]
        fill[(2, 3)] += [(0, f_qk(1, 3, 2)), (2, f_qk(0, 0, 3)),
                         (4, f_v(12)), (6, f_proj(6, 0)),
                         (8, f_proj(6, 1)), (10, f_proj(7, 0))]
        fill[(3, 0)] += [(0, f_qk(1, 0, 3)), (0, f_v(13)), (1, f_v(14)),
                         (2, f_v(15)), (3, f_qk(0, 1, 3))] + \
            [(4 + jj, f_rp(2, jj)) for jj in range(4)] + \
            [(7, f_nm(2, 0)), (9, f_nm(2, 1)), (11, f_nm(2, 2)),
             (13, f_nm(2, 3))]
        fill[(3, 1)] += [(0, f_qk(1, 1, 3)), (1, f_proj(7, 1)),
                         (2, f_proj(8, 0)), (3, f_proj(8, 1)),
                         (4, f_qk(0, 2, 3)),
                         (11, f_rp(3, 0)), (13, f_nm(3, 0))]
        fill[(3, 2)] += [(0, f_qk(1, 2, 3)), (2, f_proj(9, 0)),
                         (4, f_qk(0, 3, 3)), (6, f_proj(9, 1)),
                         (8, f_proj(10, 0)), (10, f_rp(3, 1)),
                         (12, f_nm(3, 1))]
        fill[(3, 3)] += [(0, f_qk(1, 3, 3)), (1, f_proj(10, 1)),
                         (3, f_proj(11, 0)), (6, f_rp(3, 2)),
                         (8, f_proj(11, 1)), (10, f_nm(3, 2))]

        # ---- emission ----
        # q00 and k00 ko-interleaved: both PSUM accumulations track the
        # arriving xT pieces, so both finish ~one MM after the last piece
        # lands instead of k00 trailing q00 by a whole chain
        psq = ps_mm.tile([128, 512], F32, tag="mm")
        psk = ps_mm.tile([128, 512], F32, tag="mm")
        for ko in range(8):
            nc.tensor.matmul(psq[:], lhsT=wa_q(ko, 0), rhs=xT[:, 0, ko, :],
                             start=(ko == 0), stop=(ko == 7))
            nc.tensor.matmul(psk[:], lhsT=wa_k(ko, 0), rhs=xT[:, 0, ko, :],
                             start=(ko == 0), stop=(ko == 7))
        nc.vector.tensor_tensor(
            qT[:, 0, 0:512], psq[:],
            bqk[:, 0:1].to_broadcast((128, 512)), ADD)
        nc.vector.tensor_tensor(
            kT[:, 0, 0:512], psk[:],
            bqk[:, 4:5].to_broadcast((128, 512)), ADD)
        # first scores+exp of attn(0,0) go ahead of everything else: the
        # exp stream (the global pacer) starts as soon as q00/k00 land;
        # v chains ride as (0,0) fillers, gating only the (stallable) AVs
        head0 = attn_scores(0, 0, 0)
        calls = [(c, j) for c in range(4) for j in range(4)]
        head = head0
        for ci, (c, j) in enumerate(calls):
            nxt = None
            if ci + 1 < len(calls):
                nc_, nj_ = calls[ci + 1]
                nxt = (nj_, nc_)
            head = attn(j, c, fill[(c, j)], head=head, nxt=nxt)
        # tail: only pair 3's softmax remains.  Pre-open three proj chunks
        # (j2 0-2 partials) so the PE has work while the sums DMA ->
        # reciprocal -> selector-matmul chain resolves; norm(3, 3)'s pbc
        # comes from the now-free ps_av pool so the pre-opened chains can
        # hold both ps_mm banks.
        ps0 = proj_start(12, 0, nj=3)
        ps1 = proj_start(12, 1, nj=3)
        ps2 = proj_start(13, 0, pool=ps_av, nj=3)
        recip_pair(3, 3)
        norm_mult(3, 3, pool=ps_av)
        proj_finish(12, 0, ps0, nj0=3, split_dma=True)
        proj_finish(12, 1, ps1, nj0=3, split_dma=True)
        proj_finish(13, 0, ps2, nj0=3, split_dma=True)
        proj_chain2(13, 1)
        proj_chain2(14, 0)
        proj_chain2(15, 0)
        proj_chain2(14, 1)
        proj_chain2(15, 1)

    nc.compile()
    return nc


def _prep_inputs(x, W_attn, b_attn, W_proj, b_proj):
    x = np.asarray(x, dtype=np.float32)
    W_attn = np.asarray(W_attn, dtype=np.float32)
    b_attn = np.asarray(b_attn, dtype=np.float32)
    W_proj = np.asarray(W_proj, dtype=np.float32)
    b_proj = np.asarray(b_proj, dtype=np.float32)

    bv = b_attn[2 * C:3 * C]
    bye_full = (b_proj + bv @ W_proj) * 0.5
    bye = bye_full[None, :].astype(bf16)

    # shared triangle mask for diagonal k-tiles: keep iff p <= f
    tri = (np.arange(128)[:, None, None] <= np.arange(128)[None, None, :])
    tri = tri.astype(np.float32).astype(bf16)

    # broadcast selectors: for pair j quarter i, out partition p gets
    # rsum partition 32j + 4*(p>=64) + i (pairs based at 32j so the DVE
    # reciprocal's 8-partition slice starts 32-aligned)
    sel = np.zeros((128, 16, 128), np.float32)
    for j in range(4):
        for i in range(4):
            sel[32 * j + i, 4 * j + i, 0:64] = 1.0
            sel[32 * j + 4 + i, 4 * j + i, 64:128] = 1.0
    sel = sel.astype(bf16)

    def pack_ko(w):  # [1024, n] -> [128, 8, n] per-partition contiguous
        return np.ascontiguousarray(
            w.reshape(8, 128, w.shape[1]).transpose(1, 0, 2)).astype(bf16)

    in_maps = []
    for c in range(NCORES):
        b, r = c // 2, c % 2
        xT = np.ascontiguousarray(x[b].T)  # [C, T]
        xt = np.ascontiguousarray(
            xT.reshape(8, 128, 4, 512).transpose(1, 2, 0, 3)).astype(bf16)
        qs, ks, vs = CL * r, C + CL * r, 2 * C + CL * r
        Wq = W_attn[:, qs:qs + CL]
        Wk = W_attn[:, ks:ks + CL]
        Wv = W_attn[:, vs:vs + CL]
        wp = np.ascontiguousarray(
            W_proj[CL * r:CL * (r + 1), :].reshape(4, 128, C)
            .transpose(1, 0, 2)).astype(bf16)
        bqk = np.empty((128, 8), np.float32)
        for j in range(4):
            bqk[:, j] = b_attn[qs + 128 * j:qs + 128 * (j + 1)]
            bqk[:, 4 + j] = b_attn[ks + 128 * j:ks + 128 * (j + 1)]
        in_maps.append({
            "xt": xt,
            "waq0": pack_ko(Wq[:, 0:128]), "wak0": pack_ko(Wk[:, 0:128]),
            "wav": pack_ko(Wv),
            "waq13": np.ascontiguousarray(np.stack(
                [pack_ko(Wq[:, 128 * jj:128 * (jj + 1)])
                 for jj in (1, 2, 3)], axis=1)),
            "wak13": np.ascontiguousarray(np.stack(
                [pack_ko(Wk[:, 128 * jj:128 * (jj + 1)])
                 for jj in (1, 2, 3)], axis=1)),
            "wp": wp, "bqk": bqk, "bye": np.asarray(bye),
            "tri": np.asarray(tri), "sel": np.asarray(sel)})
    return in_maps


def kernel(x, W_attn, b_attn, W_proj, b_proj, _trace=False, _result=[None]):
    if "nc" not in _CACHE:
        _CACHE["nc"] = _build()
    nc = _CACHE["nc"]
    in_maps = _prep_inputs(x, W_attn, b_attn, W_proj, b_proj)
    res = run_bass_kernel_spmd(nc, in_maps, list(range(NCORES)), trace=_trace)
    _result[0] = res
    out = np.empty((B, T, C), np.float32)
    for b in range(B):
        yc = (res.results[2 * b]["y"].astype(np.float32)
              + res.results[2 * b + 1]["y"].astype(np.float32))
        out[b] = yc.reshape(T, C)
    return out
